# revision 9
# baseline (speedup 1.0000x reference)
"""TRN2 Bass kernel for the ConceptualMambaBlock problem.

Math (reference):
    x: [B=4, T=96, N=512, H=128] f32
    expanded = x @ W_exp.T + b_exp            # [B,T,N,2H]
    primary, gating = split(expanded, 2, -1)
    s_t = 0.9*s_{t-1} + 0.1*gating_t          # EMA along T
    out = (primary * sigmoid(s)) @ W_con.T + b_con

Strategy:
  - Shard (B x N/2) over 8 cores: core c -> batch c//2, node half c%2.
  - Host pre-transposes each core's x shard to [H, N_local, T] so the
    contraction dim H lands on SBUF partitions with fully-contiguous DMA;
    no on-chip transposes anywhere.
  - Per 4-node block (tok = 4*96 = 384 columns, t fastest):
      mm1 (fp32r, full PE rate) -> PSUM gating/primary [o=128, tok]
      gating bias via K=1 accumulate-matmul (weights/bias pre-scaled by 0.1)
      EMA via DVE tensor_tensor_scan: state = mask*state + g  (mask has 0.0
      at each t=0 column, so the 4 node-segments reset exactly)
      sigmoid on ACT; gate-mul + primary bias in one DVE op;
      mm2 (fp32r); output bias via ACT Identity; DMA out.
  - Matmuls are batched by weight across groups of GRP=4 blocks so the PE
    streams N-cycle back-to-back matmuls instead of paying the isolated
    (219+N)-cycle cost on every weight switch.  mm2 of group g-1 is emitted
    inside group g (software pipeline) so the PE never waits on the current
    group's DVE/ACT chain.
  - PSUM: "pg" tag holds the gating tiles (4 banks); "pq" tag is shared by
    the primary (pp) and output (po) tiles (4 banks), whose lifetimes
    alternate.
  - DMA is grouped: one load / one store covers GRP consecutive blocks.
  - Output returned as [H, N_local, T] per core; host transposes back.
"""

import numpy as np

import concourse.bacc as bacc
import concourse.bass as bass  # noqa: F401  (engine types referenced via nc)
import concourse.mybir as mybir
import concourse.tile as tile
from concourse.bass_utils import run_bass_kernel_spmd

F32 = mybir.dt.float32
F32R = mybir.dt.float32r
AF = mybir.ActivationFunctionType
ALU = mybir.AluOpType

B, T, N, H = 4, 96, 512, 128
NCORES = 8
NLOC = N // 2          # 256 nodes per core
NB = 4                 # nodes per block
TOK = NB * T           # 384 columns per block
NBLK = NLOC // NB      # 64 blocks per core
GRP = 4                # blocks per group (DMA + matmul phase batch)
NGRP = NBLK // GRP

_NC_CACHE = None


def _build():
    nc = bacc.Bacc()

    xt_h = nc.dram_tensor("xt", [H, NBLK, TOK], F32R, kind="ExternalInput")
    w1p_h = nc.dram_tensor("w1p", [H, H], F32R, kind="ExternalInput")
    w1g_h = nc.dram_tensor("w1g", [H, H], F32R, kind="ExternalInput")
    w2_h = nc.dram_tensor("w2", [H, H], F32R, kind="ExternalInput")
    b1g_h = nc.dram_tensor("b1g", [1, H], F32R, kind="ExternalInput")
    b1p_h = nc.dram_tensor("b1p", [H, 1], F32, kind="ExternalInput")
    b2_h = nc.dram_tensor("b2", [H, 1], F32, kind="ExternalInput")
    ones_h = nc.dram_tensor("ones", [1, TOK], F32R, kind="ExternalInput")
    out_h = nc.dram_tensor("out", [H, NBLK, TOK], F32, kind="ExternalOutput")

    with tile.TileContext(nc) as tc:
        with (
            tc.tile_pool(name="consts", bufs=1) as cp,
            tc.tile_pool(name="io", bufs=3) as io,
            tc.tile_pool(name="mid", bufs=8) as mid,
            tc.tile_pool(name="ps", bufs=2, space="PSUM") as ps,
        ):
            w1p_sb = cp.tile([H, H], F32R, tag="w1p")
            nc.gpsimd.dma_start(out=w1p_sb[:], in_=w1p_h[:, :])
            w1g_sb = cp.tile([H, H], F32R, tag="w1g")
            nc.gpsimd.dma_start(out=w1g_sb[:], in_=w1g_h[:, :])
            w2_sb = cp.tile([H, H], F32R, tag="w2")
            nc.gpsimd.dma_start(out=w2_sb[:], in_=w2_h[:, :])
            b1g_sb = cp.tile([1, H], F32R, tag="b1g")
            nc.gpsimd.dma_start(out=b1g_sb[:], in_=b1g_h[:, :])
            b1p_sb = cp.tile([H, 1], F32, tag="b1p")
            nc.gpsimd.dma_start(out=b1p_sb[:], in_=b1p_h[:, :])
            b2_sb = cp.tile([H, 1], F32, tag="b2")
            nc.gpsimd.dma_start(out=b2_sb[:], in_=b2_h[:, :])
            ones_sb = cp.tile([1, TOK], F32R, tag="ones")
            nc.gpsimd.dma_start(out=ones_sb[:], in_=ones_h[:, :])

            mask_sb = cp.tile([H, NB, T], F32, tag="mask")
            nc.gpsimd.memset(mask_sb[:], 0.9)
            nc.gpsimd.memset(mask_sb[:, :, 0:1], 0.0)
            mask2d = mask_sb[:].rearrange("p a b -> p (a b)")

            # Software pipeline over MM-groups of 2 blocks with one full
            # iteration of slack on every cross-engine edge:
            #   PE (iter g) : bias(g) x2 -> w1g(g) x2 -> w1p(g) x2 -> w2(g-1) x2
            #   DVE (iter g): stt(g-1) x2 -> scan(g) x2
            #   ACT (iter g): sig(g) x2 -> id(g-1) x2
            MG = 2                    # blocks per matmul phase group
            NMG = NBLK // MG          # 32 iterations
            DG = GRP // MG            # MM-groups per DMA group

            state = {}                # per-iteration tiles carried forward

            def emit_stt(g):
                # gate-mul of iteration g (y = (pp + b1p) * sig)
                pps, sgs = state[g]["pps"], state[g]["sgs"]
                ys = []
                for j in range(MG):
                    y = mid.tile([H, TOK], F32R, tag="y", name=f"y{j}")
                    nc.vector.scalar_tensor_tensor(
                        out=y[:], in0=pps[j][:], scalar=b1p_sb[:], in1=sgs[j][:],
                        op0=ALU.add, op1=ALU.mult,
                    )
                    ys.append(y)
                state[g]["ys"] = ys

            def emit_mm2_and_out(g):
                ys, ob4 = state[g]["ys"], state[g]["ob4"]
                pos = []
                for j in range(MG):
                    po = ps.tile([H, TOK], F32, tag="po", name=f"po{j}", bufs=2)
                    nc.tensor.matmul(
                        po[:], lhsT=w2_sb[:], rhs=ys[j][:], start=True, stop=True
                    )
                    pos.append(po)
                for j in range(MG):
                    nc.scalar.activation(
                        ob4[:, (g % DG) * MG + j, :], pos[j][:],
                        AF.Identity, bias=b2_sb[:], scale=1.0,
                    )
                if g % DG == DG - 1:
                    dgi = g // DG
                    nc.gpsimd.dma_start(
                        out=out_h[:, dgi * GRP : (dgi + 1) * GRP, :], in_=ob4[:]
                    )
                del state[g]

            xt4 = None
            ob4 = None
            for g in range(NMG):
                if g % DG == 0:
                    dgi = g // DG
                    xt4 = io.tile([H, GRP, TOK], F32R, tag="xt", name="xt4")
                    nc.sync.dma_start(
                        out=xt4[:], in_=xt_h[:, dgi * GRP : (dgi + 1) * GRP, :]
                    )
                    ob4 = io.tile([H, GRP, TOK], F32, tag="ob", name="ob4")
                xts = [xt4[:, (g % DG) * MG + j, :] for j in range(MG)]
                state[g] = {"ob4": ob4}

                # PE: gating phases
                pgs = [ps.tile([H, TOK], F32, tag="pg", name=f"pg{j}", bufs=3)
                       for j in range(MG)]
                for j in range(MG):
                    nc.tensor.matmul(
                        pgs[j][:], lhsT=b1g_sb[:], rhs=ones_sb[:], start=True, stop=False
                    )
                for j in range(MG):
                    nc.tensor.matmul(
                        pgs[j][:], lhsT=w1g_sb[:], rhs=xts[j], start=False, stop=True
                    )

                # DVE: previous iteration's gate-mul first (deps long ready)
                if g - 1 in state and "sgs" in state.get(g - 1, {}):
                    emit_stt(g - 1)

                # PE: primary phase
                pps = [ps.tile([H, TOK], F32, tag="pp", name=f"pp{j}", bufs=3)
                       for j in range(MG)]
                for j in range(MG):
                    nc.tensor.matmul(
                        pps[j][:], lhsT=w1p_sb[:], rhs=xts[j], start=True, stop=True
                    )
                state[g]["pps"] = pps

                # DVE: this iteration's scans
                ss = []
                for j in range(MG):
                    s = mid.tile([H, TOK], F32, tag="s", name=f"s{j}")
                    nc.vector.tensor_tensor_scan(
                        out=s[:], data0=mask2d, data1=pgs[j][:],
                        initial=0.0, op0=ALU.mult, op1=ALU.add,
                    )
                    ss.append(s)
                # ACT: sigmoids
                sgs = []
                for j in range(MG):
                    sg = mid.tile([H, TOK], F32, tag="sg", name=f"sg{j}")
                    nc.scalar.activation(sg[:], ss[j][:], AF.Sigmoid)
                    sgs.append(sg)
                state[g]["sgs"] = sgs

                # PE: mm2 of g-1 (y produced by the stt emitted above)
                if g - 1 in state and "ys" in state.get(g - 1, {}):
                    emit_mm2_and_out(g - 1)

            # drain: stt + mm2 of the last iteration
            emit_stt(NMG - 1)
            emit_mm2_and_out(NMG - 1)

    nc.finalize()
    return nc


def _get_nc():
    global _NC_CACHE
    if _NC_CACHE is None:
        _NC_CACHE = _build()
    return _NC_CACHE


def _in_maps(x, W_exp, b_exp, W_con, b_con):
    w1p = np.ascontiguousarray(W_exp[:H, :].T, dtype=np.float32)
    w1g = np.ascontiguousarray((0.1 * W_exp[H:, :]).T, dtype=np.float32)
    w2 = np.ascontiguousarray(W_con.T, dtype=np.float32)
    b1g = np.ascontiguousarray((0.1 * b_exp[H:]).reshape(1, H), dtype=np.float32)
    b1p = np.ascontiguousarray(b_exp[:H].reshape(H, 1), dtype=np.float32)
    b2 = np.ascontiguousarray(b_con.reshape(H, 1), dtype=np.float32)

    maps = []
    for c in range(NCORES):
        bb, nh = c // 2, c % 2
        xs = x[bb, :, nh * NLOC : (nh + 1) * NLOC, :]  # [T, NLOC, H]
        xT = np.ascontiguousarray(xs.transpose(2, 1, 0)).reshape(H, NBLK, TOK)
        maps.append(
            {
                "xt": xT,
                "w1p": w1p,
                "w1g": w1g,
                "w2": w2,
                "b1g": b1g,
                "b1p": b1p,
                "b2": b2,
                "ones": np.ones((1, TOK), dtype=np.float32),
            }
        )
    return maps


def run_spmd(x, W_exp, b_exp, W_con, b_con, **spmd_kwargs):
    """Run the 8-core kernel; returns (full_output, BassKernelResults)."""
    maps = _in_maps(x, W_exp, b_exp, W_con, b_con)
    res = run_bass_kernel_spmd(
        _get_nc(), maps, core_ids=list(range(NCORES)), **spmd_kwargs
    )
    out = np.empty((B, T, N, H), dtype=np.float32)
    for c in range(NCORES):
        bb, nh = c // 2, c % 2
        oT = res.results[c]["out"].reshape(H, NLOC, T)
        out[bb, :, nh * NLOC : (nh + 1) * NLOC, :] = oT.transpose(2, 1, 0)
    return out, res


def kernel(spatial_temporal_representation, W_exp, b_exp, W_con, b_con):
    out, _ = run_spmd(
        np.asarray(spatial_temporal_representation, dtype=np.float32),
        np.asarray(W_exp, dtype=np.float32),
        np.asarray(b_exp, dtype=np.float32),
        np.asarray(W_con, dtype=np.float32),
        np.asarray(b_con, dtype=np.float32),
    )
    return out


# revision 10
# speedup vs baseline: 1.2039x; 1.2039x over previous
"""TRN2 Bass kernel for the ConceptualMambaBlock problem.

Math (reference):
    x: [B=4, T=96, N=512, H=128] f32
    expanded = x @ W_exp.T + b_exp            # [B,T,N,2H]
    primary, gating = split(expanded, 2, -1)
    s_t = 0.9*s_{t-1} + 0.1*gating_t          # EMA along T
    out = (primary * sigmoid(s)) @ W_con.T + b_con

Strategy:
  - Shard (B x N/2) over 8 cores: core c -> batch c//2, node half c%2.
  - Host pre-transposes each core's x shard to [H, N_local, T] so the
    contraction dim H lands on SBUF partitions with fully-contiguous DMA;
    no on-chip transposes anywhere.
  - Per 4-node block (tok = 4*96 = 384 columns, t fastest):
      mm1 (fp32r, full PE rate) -> PSUM gating/primary [o=128, tok]
      gating bias via K=1 accumulate-matmul (weights/bias pre-scaled by 0.1)
      EMA via DVE tensor_tensor_scan: state = mask*state + g  (mask has 0.0
      at each t=0 column, so the 4 node-segments reset exactly)
      sigmoid on ACT; gate-mul + primary bias in one DVE op;
      mm2 (fp32r); output bias via ACT Identity; DMA out.
  - Matmuls are batched by weight across groups of GRP=4 blocks so the PE
    streams N-cycle back-to-back matmuls instead of paying the isolated
    (219+N)-cycle cost on every weight switch.  mm2 of group g-1 is emitted
    inside group g (software pipeline) so the PE never waits on the current
    group's DVE/ACT chain.
  - PSUM: "pg" tag holds the gating tiles (4 banks); "pq" tag is shared by
    the primary (pp) and output (po) tiles (4 banks), whose lifetimes
    alternate.
  - DMA is grouped: one load / one store covers GRP consecutive blocks.
  - Output returned as [H, N_local, T] per core; host transposes back.
"""

import numpy as np

import concourse.bacc as bacc
import concourse.bass as bass  # noqa: F401  (engine types referenced via nc)
import concourse.mybir as mybir
import concourse.tile as tile
from concourse.bass_utils import run_bass_kernel_spmd

F32 = mybir.dt.float32
F32R = mybir.dt.float32r
AF = mybir.ActivationFunctionType
ALU = mybir.AluOpType

B, T, N, H = 4, 96, 512, 128
NCORES = 8
NLOC = N // 2          # 256 nodes per core
NB = 4                 # nodes per block
TOK = NB * T           # 384 columns per block
NBLK = NLOC // NB      # 64 blocks per core
GRP = 4                # blocks per group (DMA + matmul phase batch)
NGRP = NBLK // GRP

_NC_CACHE = None


def _build():
    nc = bacc.Bacc()

    xt_h = nc.dram_tensor("xt", [H, NBLK, TOK], F32R, kind="ExternalInput")
    w1p_h = nc.dram_tensor("w1p", [H, H], F32R, kind="ExternalInput")
    w1g_h = nc.dram_tensor("w1g", [H, H], F32R, kind="ExternalInput")
    w2_h = nc.dram_tensor("w2", [H, H], F32R, kind="ExternalInput")
    bneg_h = nc.dram_tensor("bneg", [H, 1], F32, kind="ExternalInput")
    bg_h = nc.dram_tensor("bg", [H, 1], F32, kind="ExternalInput")
    b1p_h = nc.dram_tensor("b1p", [H, 1], F32, kind="ExternalInput")
    b2_h = nc.dram_tensor("b2", [H, 1], F32, kind="ExternalInput")
    out_h = nc.dram_tensor("out", [H, NBLK, TOK], F32, kind="ExternalOutput")

    with tile.TileContext(nc) as tc:
        with (
            tc.tile_pool(name="consts", bufs=1) as cp,
            tc.tile_pool(name="io", bufs=3) as io,
            tc.tile_pool(name="mid", bufs=8) as mid,
            tc.tile_pool(name="ps", bufs=2, space="PSUM") as ps,
        ):
            w1p_sb = cp.tile([H, H], F32R, tag="w1p")
            nc.gpsimd.dma_start(out=w1p_sb[:], in_=w1p_h[:, :])
            w1g_sb = cp.tile([H, H], F32R, tag="w1g")
            nc.gpsimd.dma_start(out=w1g_sb[:], in_=w1g_h[:, :])
            w2_sb = cp.tile([H, H], F32R, tag="w2")
            nc.gpsimd.dma_start(out=w2_sb[:], in_=w2_h[:, :])
            bneg_sb = cp.tile([H, 1], F32, tag="bneg")
            nc.gpsimd.dma_start(out=bneg_sb[:], in_=bneg_h[:, :])
            bg_sb = cp.tile([H, 1], F32, tag="bg")
            nc.gpsimd.dma_start(out=bg_sb[:], in_=bg_h[:, :])
            b1p_sb = cp.tile([H, 1], F32, tag="b1p")
            nc.gpsimd.dma_start(out=b1p_sb[:], in_=b1p_h[:, :])
            b2_sb = cp.tile([H, 1], F32, tag="b2")
            nc.gpsimd.dma_start(out=b2_sb[:], in_=b2_h[:, :])

            mask_sb = cp.tile([H, NB, T], F32, tag="mask")
            nc.gpsimd.memset(mask_sb[:], 0.9)
            nc.gpsimd.memset(mask_sb[:, :, 0:1], 0.0)
            mask2d = mask_sb[:].rearrange("p a b -> p (a b)")

            # Software pipeline over MM-groups of 2 blocks with one full
            # iteration of slack on every cross-engine edge:
            #   PE (iter g) : bias(g) x2 -> w1g(g) x2 -> w1p(g) x2 -> w2(g-1) x2
            #   DVE (iter g): stt(g-1) x2 -> scan(g) x2
            #   ACT (iter g): sig(g) x2 -> id(g-1) x2
            MG = 2                    # blocks per matmul phase group
            NMG = NBLK // MG          # 32 iterations
            DG = GRP // MG            # MM-groups per DMA group

            state = {}                # per-iteration tiles carried forward

            def emit_stt(g):
                # gate-mul of iteration g (y = (pp + b1p) * sig)
                pps, sgs = state[g]["pps"], state[g]["sgs"]
                ys = []
                for j in range(MG):
                    y = mid.tile([H, TOK], F32R, tag="y", name=f"y{j}")
                    nc.vector.scalar_tensor_tensor(
                        out=y[:], in0=pps[j][:], scalar=b1p_sb[:], in1=sgs[j][:],
                        op0=ALU.add, op1=ALU.mult,
                    )
                    ys.append(y)
                state[g]["ys"] = ys

            def emit_mm2_and_out(g):
                ys, ob4 = state[g]["ys"], state[g]["ob4"]
                pos = []
                for j in range(MG):
                    po = ps.tile([H, TOK], F32, tag="po", name=f"po{j}", bufs=2)
                    nc.tensor.matmul(
                        po[:], lhsT=w2_sb[:], rhs=ys[j][:], start=True, stop=True
                    )
                    pos.append(po)
                for j in range(MG):
                    nc.scalar.activation(
                        ob4[:, (g % DG) * MG + j, :], pos[j][:],
                        AF.Identity, bias=b2_sb[:], scale=1.0,
                    )
                if g % DG == DG - 1:
                    dgi = g // DG
                    nc.gpsimd.dma_start(
                        out=out_h[:, dgi * GRP : (dgi + 1) * GRP, :], in_=ob4[:]
                    )
                del state[g]

            xt4 = None
            ob4 = None
            for g in range(NMG):
                if g % DG == 0:
                    dgi = g // DG
                    xt4 = io.tile([H, GRP, TOK], F32R, tag="xt", name="xt4")
                    nc.sync.dma_start(
                        out=xt4[:], in_=xt_h[:, dgi * GRP : (dgi + 1) * GRP, :]
                    )
                    ob4 = io.tile([H, GRP, TOK], F32, tag="ob", name="ob4")
                xts = [xt4[:, (g % DG) * MG + j, :] for j in range(MG)]
                state[g] = {"ob4": ob4}

                # PE: gating phases
                pgs = [ps.tile([H, TOK], F32, tag="pg", name=f"pg{j}", bufs=3)
                       for j in range(MG)]
                for j in range(MG):
                    nc.tensor.matmul(
                        pgs[j][:], lhsT=w1g_sb[:], rhs=xts[j], start=True, stop=True
                    )
                # sigma-shift fixup: t=0 columns of the gating PSUM get -0.9*b_g
                # (the remaining +b_g shift is folded into the sigmoid bias)
                for j in range(MG):
                    pgc = pgs[j][:].rearrange("p (a b) -> p a b", b=T)[:, :, 0:1]
                    nc.scalar.activation(pgc, pgc, AF.Identity, bias=bneg_sb[:], scale=1.0)

                # DVE: previous iteration's gate-mul first (deps long ready)
                if g - 1 in state and "sgs" in state.get(g - 1, {}):
                    emit_stt(g - 1)

                # PE: primary phase
                pps = [ps.tile([H, TOK], F32, tag="pp", name=f"pp{j}", bufs=3)
                       for j in range(MG)]
                for j in range(MG):
                    nc.tensor.matmul(
                        pps[j][:], lhsT=w1p_sb[:], rhs=xts[j], start=True, stop=True
                    )
                state[g]["pps"] = pps

                # DVE: this iteration's scans
                ss = []
                for j in range(MG):
                    s = mid.tile([H, TOK], F32, tag="s", name=f"s{j}")
                    nc.vector.tensor_tensor_scan(
                        out=s[:], data0=mask2d, data1=pgs[j][:],
                        initial=0.0, op0=ALU.mult, op1=ALU.add,
                    )
                    ss.append(s)
                # ACT: sigmoids
                sgs = []
                for j in range(MG):
                    sg = mid.tile([H, TOK], F32, tag="sg", name=f"sg{j}")
                    nc.scalar.activation(sg[:], ss[j][:], AF.Sigmoid, bias=bg_sb[:], scale=1.0)
                    sgs.append(sg)
                state[g]["sgs"] = sgs

                # PE: mm2 of g-1 (y produced by the stt emitted above)
                if g - 1 in state and "ys" in state.get(g - 1, {}):
                    emit_mm2_and_out(g - 1)

            # drain: stt + mm2 of the last iteration
            emit_stt(NMG - 1)
            emit_mm2_and_out(NMG - 1)

    nc.finalize()
    return nc


def _get_nc():
    global _NC_CACHE
    if _NC_CACHE is None:
        _NC_CACHE = _build()
    return _NC_CACHE


def _in_maps(x, W_exp, b_exp, W_con, b_con):
    w1p = np.ascontiguousarray(W_exp[:H, :].T, dtype=np.float32)
    w1g = np.ascontiguousarray((0.1 * W_exp[H:, :]).T, dtype=np.float32)
    w2 = np.ascontiguousarray(W_con.T, dtype=np.float32)
    bneg = np.ascontiguousarray((-0.9 * b_exp[H:]).reshape(H, 1), dtype=np.float32)
    bgv = np.ascontiguousarray(b_exp[H:].reshape(H, 1), dtype=np.float32)
    b1p = np.ascontiguousarray(b_exp[:H].reshape(H, 1), dtype=np.float32)
    b2 = np.ascontiguousarray(b_con.reshape(H, 1), dtype=np.float32)

    maps = []
    for c in range(NCORES):
        bb, nh = c // 2, c % 2
        xs = x[bb, :, nh * NLOC : (nh + 1) * NLOC, :]  # [T, NLOC, H]
        xT = np.ascontiguousarray(xs.transpose(2, 1, 0)).reshape(H, NBLK, TOK)
        maps.append(
            {
                "xt": xT,
                "w1p": w1p,
                "w1g": w1g,
                "w2": w2,
                "bneg": bneg,
                "bg": bgv,
                "b1p": b1p,
                "b2": b2,
            }
        )
    return maps


def run_spmd(x, W_exp, b_exp, W_con, b_con, **spmd_kwargs):
    """Run the 8-core kernel; returns (full_output, BassKernelResults)."""
    maps = _in_maps(x, W_exp, b_exp, W_con, b_con)
    res = run_bass_kernel_spmd(
        _get_nc(), maps, core_ids=list(range(NCORES)), **spmd_kwargs
    )
    out = np.empty((B, T, N, H), dtype=np.float32)
    for c in range(NCORES):
        bb, nh = c // 2, c % 2
        oT = res.results[c]["out"].reshape(H, NLOC, T)
        out[bb, :, nh * NLOC : (nh + 1) * NLOC, :] = oT.transpose(2, 1, 0)
    return out, res


def kernel(spatial_temporal_representation, W_exp, b_exp, W_con, b_con):
    out, _ = run_spmd(
        np.asarray(spatial_temporal_representation, dtype=np.float32),
        np.asarray(W_exp, dtype=np.float32),
        np.asarray(b_exp, dtype=np.float32),
        np.asarray(W_con, dtype=np.float32),
        np.asarray(b_con, dtype=np.float32),
    )
    return out


# revision 11
# speedup vs baseline: 1.2467x; 1.0355x over previous
"""TRN2 Bass kernel for the ConceptualMambaBlock problem.

Math (reference):
    x: [B=4, T=96, N=512, H=128] f32
    expanded = x @ W_exp.T + b_exp            # [B,T,N,2H]
    primary, gating = split(expanded, 2, -1)
    s_t = 0.9*s_{t-1} + 0.1*gating_t          # EMA along T
    out = (primary * sigmoid(s)) @ W_con.T + b_con

Strategy:
  - Shard (B x N/2) over 8 cores: core c -> batch c//2, node half c%2.
  - Host pre-transposes each core's x shard to [H, N_local, T] so the
    contraction dim H lands on SBUF partitions with fully-contiguous DMA;
    no on-chip transposes anywhere.
  - Per 4-node block (tok = 4*96 = 384 columns, t fastest):
      mm1 (fp32r, full PE rate) -> PSUM gating/primary [o=128, tok]
      gating bias via K=1 accumulate-matmul (weights/bias pre-scaled by 0.1)
      EMA via DVE tensor_tensor_scan: state = mask*state + g  (mask has 0.0
      at each t=0 column, so the 4 node-segments reset exactly)
      sigmoid on ACT; gate-mul + primary bias in one DVE op;
      mm2 (fp32r); output bias via ACT Identity; DMA out.
  - Matmuls are batched by weight across groups of GRP=4 blocks so the PE
    streams N-cycle back-to-back matmuls instead of paying the isolated
    (219+N)-cycle cost on every weight switch.  mm2 of group g-1 is emitted
    inside group g (software pipeline) so the PE never waits on the current
    group's DVE/ACT chain.
  - PSUM: "pg" tag holds the gating tiles (4 banks); "pq" tag is shared by
    the primary (pp) and output (po) tiles (4 banks), whose lifetimes
    alternate.
  - DMA is grouped: one load / one store covers GRP consecutive blocks.
  - Output returned as [H, N_local, T] per core; host transposes back.
"""

import numpy as np

import concourse.bacc as bacc
import concourse.bass as bass  # noqa: F401  (engine types referenced via nc)
import concourse.mybir as mybir
import concourse.tile as tile
from concourse.bass_utils import run_bass_kernel_spmd

F32 = mybir.dt.float32
F32R = mybir.dt.float32r
AF = mybir.ActivationFunctionType
ALU = mybir.AluOpType

B, T, N, H = 4, 96, 512, 128
NCORES = 8
NLOC = N // 2          # 256 nodes per core
NB = 4                 # nodes per block
TOK = NB * T           # 384 columns per block
NBLK = NLOC // NB      # 64 blocks per core
GRP = 4                # blocks per group (DMA + matmul phase batch)
NGRP = NBLK // GRP

_NC_CACHE = None


def _build():
    nc = bacc.Bacc()

    xt_h = nc.dram_tensor("xt", [H, NBLK, TOK], F32R, kind="ExternalInput")
    wpack_h = nc.dram_tensor("wpack", [H, 3 * H], F32R, kind="ExternalInput")
    bpack_h = nc.dram_tensor("bpack", [H, 4], F32, kind="ExternalInput")
    out_h = nc.dram_tensor("out", [H, NBLK, TOK], F32, kind="ExternalOutput")

    with tile.TileContext(nc) as tc:
        with (
            tc.tile_pool(name="consts", bufs=1) as cp,
            tc.tile_pool(name="io", bufs=3) as io,
            tc.tile_pool(name="mid", bufs=8) as mid,
            tc.tile_pool(name="ps", bufs=2, space="PSUM") as ps,
        ):
            wpack_sb = cp.tile([H, 3 * H], F32R, tag="wpack")
            nc.sync.dma_start(out=wpack_sb[:], in_=wpack_h[:, :])
            bpack_sb = cp.tile([H, 4], F32, tag="bpack")
            nc.sync.dma_start(out=bpack_sb[:], in_=bpack_h[:, :])
            w1p_sb = wpack_sb[:, 0:H]
            w1g_sb = wpack_sb[:, H : 2 * H]
            w2_sb = wpack_sb[:, 2 * H : 3 * H]
            bneg_sb = bpack_sb[:, 0:1]
            bg_sb = bpack_sb[:, 1:2]
            b1p_sb = bpack_sb[:, 2:3]
            b2_sb = bpack_sb[:, 3:4]

            mask_sb = cp.tile([H, NB, T], F32, tag="mask")
            nc.gpsimd.memset(mask_sb[:], 0.9)
            nc.gpsimd.memset(mask_sb[:, :, 0:1], 0.0)
            mask2d = mask_sb[:].rearrange("p a b -> p (a b)")

            # Software pipeline over MM-groups of 2 blocks with one full
            # iteration of slack on every cross-engine edge:
            #   PE (iter g) : bias(g) x2 -> w1g(g) x2 -> w1p(g) x2 -> w2(g-1) x2
            #   DVE (iter g): stt(g-1) x2 -> scan(g) x2
            #   ACT (iter g): sig(g) x2 -> id(g-1) x2
            MG = 2                    # blocks per matmul phase group
            NMG = NBLK // MG          # 32 iterations
            DG = GRP // MG            # MM-groups per DMA group

            state = {}                # per-iteration tiles carried forward

            def emit_stt(g):
                # gate-mul of iteration g (y = (pp + b1p) * sig)
                pps, sgs = state[g]["pps"], state[g]["sgs"]
                ys = []
                for j in range(MG):
                    y = mid.tile([H, TOK], F32R, tag="y", name=f"y{j}")
                    nc.vector.scalar_tensor_tensor(
                        out=y[:], in0=pps[j][:], scalar=b1p_sb, in1=sgs[j][:],
                        op0=ALU.add, op1=ALU.mult,
                    )
                    ys.append(y)
                state[g]["ys"] = ys

            def emit_mm2_and_out(g):
                ys, ob4 = state[g]["ys"], state[g]["ob4"]
                pos = []
                for j in range(MG):
                    po = ps.tile([H, TOK], F32, tag="po", name=f"po{j}", bufs=2)
                    nc.tensor.matmul(
                        po[:], lhsT=w2_sb, rhs=ys[j][:], start=True, stop=True
                    )
                    pos.append(po)
                for j in range(MG):
                    nc.scalar.activation(
                        ob4[:, (g % DG) * MG + j, :], pos[j][:],
                        AF.Identity, bias=b2_sb, scale=1.0,
                    )
                if g % DG == DG - 1:
                    dgi = g // DG
                    nc.gpsimd.dma_start(
                        out=out_h[:, dgi * GRP : (dgi + 1) * GRP, :], in_=ob4[:]
                    )
                del state[g]

            xt4 = None
            ob4 = None
            for g in range(NMG):
                if g % DG == 0:
                    dgi = g // DG
                    xt4 = io.tile([H, GRP, TOK], F32R, tag="xt", name="xt4")
                    if g == 0:
                        half = GRP // 2
                        nc.sync.dma_start(
                            out=xt4[:, :half, :], in_=xt_h[:, :half, :]
                        )
                        nc.sync.dma_start(
                            out=xt4[:, half:, :], in_=xt_h[:, half:GRP, :]
                        )
                    else:
                        nc.sync.dma_start(
                            out=xt4[:], in_=xt_h[:, dgi * GRP : (dgi + 1) * GRP, :]
                        )
                    ob4 = io.tile([H, GRP, TOK], F32, tag="ob", name="ob4")
                xts = [xt4[:, (g % DG) * MG + j, :] for j in range(MG)]
                state[g] = {"ob4": ob4}

                # PE: gating phases
                pgs = [ps.tile([H, TOK], F32, tag="pg", name=f"pg{j}", bufs=3)
                       for j in range(MG)]
                for j in range(MG):
                    nc.tensor.matmul(
                        pgs[j][:], lhsT=w1g_sb, rhs=xts[j], start=True, stop=True
                    )
                # sigma-shift fixup: t=0 columns of the gating PSUM get -0.9*b_g
                # (the remaining +b_g shift is folded into the sigmoid bias)
                for j in range(MG):
                    pgc = pgs[j][:].rearrange("p (a b) -> p a b", b=T)[:, :, 0:1]
                    nc.scalar.activation(pgc, pgc, AF.Identity, bias=bneg_sb, scale=1.0)

                # DVE: previous iteration's gate-mul first (deps long ready)
                if g - 1 in state and "sgs" in state.get(g - 1, {}):
                    emit_stt(g - 1)

                # PE: primary phase
                pps = [ps.tile([H, TOK], F32, tag="pp", name=f"pp{j}", bufs=3)
                       for j in range(MG)]
                for j in range(MG):
                    nc.tensor.matmul(
                        pps[j][:], lhsT=w1p_sb, rhs=xts[j], start=True, stop=True
                    )
                state[g]["pps"] = pps

                # DVE: this iteration's scans
                ss = []
                for j in range(MG):
                    s = mid.tile([H, TOK], F32, tag="s", name=f"s{j}")
                    nc.vector.tensor_tensor_scan(
                        out=s[:], data0=mask2d, data1=pgs[j][:],
                        initial=0.0, op0=ALU.mult, op1=ALU.add,
                    )
                    ss.append(s)
                # ACT: sigmoids
                sgs = []
                for j in range(MG):
                    sg = mid.tile([H, TOK], F32, tag="sg", name=f"sg{j}")
                    nc.scalar.activation(sg[:], ss[j][:], AF.Sigmoid, bias=bg_sb, scale=1.0)
                    sgs.append(sg)
                state[g]["sgs"] = sgs

                # PE: mm2 of g-1 (y produced by the stt emitted above)
                if g - 1 in state and "ys" in state.get(g - 1, {}):
                    emit_mm2_and_out(g - 1)

            # drain: stt + mm2 of the last iteration
            emit_stt(NMG - 1)
            emit_mm2_and_out(NMG - 1)

    nc.finalize()
    return nc


def _get_nc():
    global _NC_CACHE
    if _NC_CACHE is None:
        _NC_CACHE = _build()
    return _NC_CACHE


def _in_maps(x, W_exp, b_exp, W_con, b_con):
    wpack = np.concatenate(
        [W_exp[:H, :].T, (0.1 * W_exp[H:, :]).T, W_con.T], axis=1
    ).astype(np.float32)
    wpack = np.ascontiguousarray(wpack)
    bpack = np.stack(
        [-0.9 * b_exp[H:], b_exp[H:], b_exp[:H], b_con], axis=1
    ).astype(np.float32)
    bpack = np.ascontiguousarray(bpack)

    maps = []
    for c in range(NCORES):
        bb, nh = c // 2, c % 2
        xs = x[bb, :, nh * NLOC : (nh + 1) * NLOC, :]  # [T, NLOC, H]
        xT = np.ascontiguousarray(xs.transpose(2, 1, 0)).reshape(H, NBLK, TOK)
        maps.append(
            {
                "xt": xT,
                "wpack": wpack,
                "bpack": bpack,
            }
        )
    return maps


def run_spmd(x, W_exp, b_exp, W_con, b_con, **spmd_kwargs):
    """Run the 8-core kernel; returns (full_output, BassKernelResults)."""
    maps = _in_maps(x, W_exp, b_exp, W_con, b_con)
    res = run_bass_kernel_spmd(
        _get_nc(), maps, core_ids=list(range(NCORES)), **spmd_kwargs
    )
    out = np.empty((B, T, N, H), dtype=np.float32)
    for c in range(NCORES):
        bb, nh = c // 2, c % 2
        oT = res.results[c]["out"].reshape(H, NLOC, T)
        out[bb, :, nh * NLOC : (nh + 1) * NLOC, :] = oT.transpose(2, 1, 0)
    return out, res


def kernel(spatial_temporal_representation, W_exp, b_exp, W_con, b_con):
    out, _ = run_spmd(
        np.asarray(spatial_temporal_representation, dtype=np.float32),
        np.asarray(W_exp, dtype=np.float32),
        np.asarray(b_exp, dtype=np.float32),
        np.asarray(W_con, dtype=np.float32),
        np.asarray(b_con, dtype=np.float32),
    )
    return out
